# revision 8
# baseline (speedup 1.0000x reference)
"""MoE layer (E=8 experts, top-2) on 8 Trainium2 NeuronCores.

Strategy (expert parallelism, per the sharding hint):
  - Host computes the tiny router (logits -> softmax -> top-2; 0.07% of
    total FLOPs) exactly as the jax reference does, then dispatches
    ("all-to-all" done host-side): tokens routed to expert e are gathered,
    transposed to feature-major, padded to capacity C and sent to core e
    along with that expert's weights.
  - Core e runs the expert FFN dense on its gathered tokens:
        yT = (W2.T @ gelu(W1.T @ xT + b1) + b2) * gate
    as a 2-stage tiled matmul pipeline (feature-major activations so the
    contraction dim is always on SBUF partitions; no transposes on device).
  - Host scatter-adds the 8 partial outputs back to [B, S, D].

Shapes (hardcoded): x [2, 2048, 768], W1 [8, 768, 3072], W2 [8, 3072, 768],
Wr [768, 8]. Tokens T = 4096, per-expert expected load = T*K/E = 1024,
capacity C = 1088 (seed-0 max count is 1084; host fallback handles any
overflow exactly, so capacity misses cost wall-time, never correctness).
"""

import sys

import numpy as np

sys.path.insert(0, "/opt/trn_rl_repo")

import ml_dtypes  # noqa: E402

import concourse.bacc as bacc  # noqa: E402
import concourse.bass as bass  # noqa: E402
import concourse.mybir as mybir  # noqa: E402
import concourse.tile as tile  # noqa: E402
from concourse.bass_utils import run_bass_kernel_spmd  # noqa: E402

E = 8
KTOP = 2
D = 768
I = 3072
B, S = 2, 2048
T = B * S
C = 1088          # per-expert token capacity (padded)
NCHUNK = 272      # moving free dim per matmul (<= 512 fp32 psum bank)
NCH = C // NCHUNK
DK = D // 128     # 6 contraction tiles for matmul 1
IK = I // 128     # 24 contraction tiles for matmul 2
N_CORES = 8
N_WARMUP_MM = 6   # dummy matmuls to ramp the PE HAM clock during DMA wait
W1BLK = 256       # W1 column-block tile width (streams in consumption order)
W1NB = I // W1BLK

MM_DT = mybir.dt.bfloat16
MM_NP = ml_dtypes.bfloat16

# Module-level knobs for test harness introspection.
TRACE = False
LAST_RESULT = None


def build_nc(act_func=None):
    """Build + compile the per-core Bass program (same program on all 8
    cores; per-core data differs)."""
    if act_func is None:
        act_func = mybir.ActivationFunctionType.Gelu

    nc = bacc.Bacc(
        "TRN2",
        target_bir_lowering=False,
        debug=False,
        enable_asserts=True,
        num_devices=N_CORES,
    )

    xT = nc.dram_tensor("xT", [D, C], MM_DT, kind="ExternalInput").ap()
    W1 = nc.dram_tensor("W1", [D, I], MM_DT, kind="ExternalInput").ap()
    W2 = nc.dram_tensor("W2", [I, D], MM_DT, kind="ExternalInput").ap()
    b1t = nc.dram_tensor("b1t", [128, IK], mybir.dt.float32, kind="ExternalInput").ap()
    b2t = nc.dram_tensor("b2t", [128, DK], mybir.dt.float32, kind="ExternalInput").ap()
    gb = nc.dram_tensor("gb", [128, C], mybir.dt.float32, kind="ExternalInput").ap()
    yT = nc.dram_tensor("yT", [D, C], mybir.dt.float32, kind="ExternalOutput").ap()

    with tile.TileContext(nc) as tc:
        with (
            tc.tile_pool(name="wpool", bufs=1) as wpool,
            tc.tile_pool(name="xpool", bufs=1) as xpool,
            tc.tile_pool(name="hpool", bufs=1) as hpool,
            tc.tile_pool(name="ypool", bufs=4) as ypool,
            tc.tile_pool(name="psum", bufs=8, space="PSUM") as psum_pool,
        ):
            # ---- PE warmup: ramp the HAM clock gate while DMAs land --------
            wu_w = wpool.tile([128, 128], MM_DT, name="wu_w", tag="wu_w")
            wu_r = wpool.tile([128, 512], MM_DT, name="wu_r", tag="wu_r")
            nc.gpsimd.memset(wu_w[:], 0.0)
            nc.gpsimd.memset(wu_r[:], 0.0)
            wu_ps = psum_pool.tile([128, 512], mybir.dt.float32,
                                   name="wu_ps", tag="ps")
            for _ in range(N_WARMUP_MM):
                nc.tensor.matmul(wu_ps[:], wu_w[:], wu_r[:], start=True, stop=True)

            # ---- resident loads --------------------------------------------
            # Issue in consumption order, alternating between the Sync and
            # Scalar HWDGE rings (each DMA issue occupies its ring ~0.6us):
            # stage-1 consumes x fully during i=0 and W1 column-major at
            # ~2.8us per 128-col block, so x tiles go first interleaved with
            # W1's first column blocks, then the remaining W1 blocks stream
            # j-major. W2/g load last and hide behind stage-1 compute.
            _ring = [nc.sync, nc.scalar]

            def dma(k, dst, src):
                _ring[k % 2].dma_start(dst, src)

            xsb, w1sb = [], [[None] * W1NB for _ in range(DK)]
            for kd in range(DK):
                tx = xpool.tile([128, C], MM_DT, name=f"x_{kd}", tag=f"x_{kd}")
                nc.sync.dma_start(tx[:], xT[kd * 128:(kd + 1) * 128, :])
                xsb.append(tx)
                tw = wpool.tile([128, W1BLK], MM_DT,
                                name=f"w1_{kd}_0", tag=f"w1_{kd}_0")
                nc.scalar.dma_start(tw[:], W1[kd * 128:(kd + 1) * 128, 0:W1BLK])
                w1sb[kd][0] = tw
            b1sb = wpool.tile([128, IK], mybir.dt.float32, name="b1sb", tag="b1sb")
            nc.sync.dma_start(b1sb[:], b1t[:])
            k = 1
            for j in range(1, W1NB):
                for kd in range(DK):
                    tw = wpool.tile([128, W1BLK], MM_DT,
                                    name=f"w1_{kd}_{j}", tag=f"w1_{kd}_{j}")
                    dma(k, tw[:], W1[kd * 128:(kd + 1) * 128,
                                     j * W1BLK:(j + 1) * W1BLK])
                    w1sb[kd][j] = tw
                    k += 1
            w2sb = []
            for ki in range(IK):
                t = wpool.tile([128, D], MM_DT, name=f"w2_{ki}", tag=f"w2_{ki}")
                dma(k, t[:], W2[ki * 128:(ki + 1) * 128, :])
                w2sb.append(t)
                k += 1
            b2sb = wpool.tile([128, DK], mybir.dt.float32, name="b2sb", tag="b2sb")
            nc.sync.dma_start(b2sb[:], b2t[:])
            gsb = xpool.tile([128, C], mybir.dt.float32, name="gsb", tag="gsb")
            nc.scalar.dma_start(gsb[:], gb[:])

            IPB = W1BLK // 128  # i-tiles per W1 block

            def w1_block(kd, i):
                return w1sb[kd][i // IPB][:, (i % IPB) * 128:(i % IPB + 1) * 128]

            # ---- stage 1: hT[i] = gelu(sum_d W1[d,i].T @ xT[d] + b1[i]) ----
            hsb = [
                hpool.tile([128, C], MM_DT, name=f"h_{i}", tag=f"h_{i}")
                for i in range(IK)
            ]
            for i in range(IK):
                for c in range(NCH):
                    ps = psum_pool.tile(
                        [128, NCHUNK], mybir.dt.float32,
                        name=f"ps1_{i}_{c}", tag="ps",
                    )
                    for d in range(DK):
                        nc.tensor.matmul(
                            ps[:],
                            w1_block(d, i),
                            xsb[d][:, c * NCHUNK:(c + 1) * NCHUNK],
                            start=(d == 0),
                            stop=(d == DK - 1),
                        )
                    nc.scalar.activation(
                        hsb[i][:, c * NCHUNK:(c + 1) * NCHUNK],
                        ps[:],
                        act_func,
                        bias=b1sb[:, i:i + 1],
                    )

            # ---- stage 2: yT[d] = (sum_ki W2[ki,d].T @ hT[ki] + b2[d]) * g --
            for dd in range(DK):
                for c in range(NCH):
                    ps = psum_pool.tile(
                        [128, NCHUNK], mybir.dt.float32,
                        name=f"ps2_{dd}_{c}", tag="ps",
                    )
                    for ki in range(IK):
                        nc.tensor.matmul(
                            ps[:],
                            w2sb[ki][:, dd * 128:(dd + 1) * 128],
                            hsb[ki][:, c * NCHUNK:(c + 1) * NCHUNK],
                            start=(ki == 0),
                            stop=(ki == IK - 1),
                        )
                    yt = ypool.tile(
                        [128, NCHUNK], mybir.dt.float32, name=f"y_{dd}_{c}", tag="y"
                    )
                    nc.vector.scalar_tensor_tensor(
                        yt[:],
                        ps[:],
                        b2sb[:, dd:dd + 1],
                        gsb[:, c * NCHUNK:(c + 1) * NCHUNK],
                        mybir.AluOpType.add,
                        mybir.AluOpType.mult,
                    )
                    nc.sync.dma_start(
                        yT[dd * 128:(dd + 1) * 128, c * NCHUNK:(c + 1) * NCHUNK],
                        yt[:],
                    )

    nc.compile()
    return nc


_COMPILED_NC = None


def _get_nc():
    global _COMPILED_NC
    if _COMPILED_NC is None:
        _COMPILED_NC = build_nc()
    return _COMPILED_NC


def _route(xf, Wr, br):
    """Router: logits -> softmax -> top-2. Uses jax on CPU so it is
    bit-identical to the reference; numpy fallback otherwise."""
    try:
        import jax
        import jax.numpy as jnp

        cpu = jax.devices("cpu")[0]
        with jax.default_device(cpu):
            logits = jnp.asarray(xf) @ jnp.asarray(Wr) + jnp.asarray(br)
            gates = jax.nn.softmax(logits, axis=-1)
            top_g, top_i = jax.lax.top_k(gates, KTOP)
        return np.asarray(top_g), np.asarray(top_i)
    except Exception:
        logits = xf @ np.asarray(Wr, np.float32) + np.asarray(br, np.float32)
        m = logits.max(axis=-1, keepdims=True)
        eg = np.exp(logits - m)
        gates = eg / eg.sum(axis=-1, keepdims=True)
        top_i = np.argsort(-gates, axis=-1, kind="stable")[:, :KTOP]
        top_g = np.take_along_axis(gates, top_i, axis=-1)
        return top_g.astype(np.float32), top_i.astype(np.int32)


def _host_expert(xg, W1e, b1e, W2e, b2e):
    """Exact fp32 expert FFN on host (overflow fallback only)."""
    h = xg @ W1e + b1e
    try:
        import jax

        h = np.asarray(jax.nn.gelu(h, approximate=False))
    except Exception:
        import math

        erf = np.vectorize(math.erf)
        h = 0.5 * h * (1.0 + erf(h / np.sqrt(2.0)))
    return h @ W2e + b2e


def kernel(x, W1, b1, W2, b2, Wr, br):
    global LAST_RESULT

    x = np.asarray(x, np.float32)
    W1 = np.asarray(W1, np.float32)
    b1 = np.asarray(b1, np.float32)
    W2 = np.asarray(W2, np.float32)
    b2 = np.asarray(b2, np.float32)
    Wr = np.asarray(Wr, np.float32)
    br = np.asarray(br, np.float32)

    xf = x.reshape(T, D)
    top_g, top_i = _route(xf, Wr, br)

    idxs, overflow = [], []
    in_maps = []
    for e in range(E):
        tok, kk = np.where(top_i == e)
        g = top_g[tok, kk].astype(np.float32)
        if len(tok) > C:
            overflow.append((e, tok[C:], g[C:]))
            tok, g = tok[:C], g[:C]
        idxs.append(tok)
        n = len(tok)

        xTg = np.zeros((D, C), MM_NP)
        xTg[:, :n] = xf[tok].T.astype(MM_NP)
        gbc = np.zeros((128, C), np.float32)
        gbc[:, :n] = g[None, :]
        in_maps.append({
            "xT": xTg,
            "W1": W1[e].astype(MM_NP),
            "W2": W2[e].astype(MM_NP),
            "b1t": np.ascontiguousarray(b1[e].reshape(IK, 128).T.astype(np.float32)),
            "b2t": np.ascontiguousarray(b2[e].reshape(DK, 128).T.astype(np.float32)),
            "gb": gbc,
        })

    nc = _get_nc()
    res = run_bass_kernel_spmd(nc, in_maps, list(range(N_CORES)), trace=TRACE)
    LAST_RESULT = res

    out = np.zeros((T, D), np.float32)
    for e in range(E):
        yTe = np.asarray(res.results[e]["yT"])  # [D, C] fp32
        n = len(idxs[e])
        if n:
            out[idxs[e]] += yTe[:, :n].T
    for e, tok, g in overflow:
        y = _host_expert(xf[tok], W1[e], b1[e], W2[e], b2[e])
        out[tok] += g[:, None] * y

    return out.reshape(B, S, D)


# revision 10
# speedup vs baseline: 1.2155x; 1.2155x over previous
"""MoE layer (E=8 experts, top-2) on 8 Trainium2 NeuronCores.

Strategy (expert parallelism, per the sharding hint):
  - Host computes the tiny router (logits -> softmax -> top-2; 0.07% of
    total FLOPs) exactly as the jax reference does, then dispatches
    ("all-to-all" done host-side): tokens routed to expert e are gathered,
    transposed to feature-major, padded to capacity C and sent to core e
    along with that expert's weights.
  - Core e runs the expert FFN dense on its gathered tokens:
        yT = (W2.T @ gelu(W1.T @ xT + b1) + b2) * gate
    as a 2-stage tiled matmul pipeline (feature-major activations so the
    contraction dim is always on SBUF partitions; no transposes on device).
  - Host scatter-adds the 8 partial outputs back to [B, S, D].

Shapes (hardcoded): x [2, 2048, 768], W1 [8, 768, 3072], W2 [8, 3072, 768],
Wr [768, 8]. Tokens T = 4096, per-expert expected load = T*K/E = 1024,
capacity C = 1088 (seed-0 max count is 1084; host fallback handles any
overflow exactly, so capacity misses cost wall-time, never correctness).
"""

import sys

import numpy as np

sys.path.insert(0, "/opt/trn_rl_repo")

import ml_dtypes  # noqa: E402

import concourse.bacc as bacc  # noqa: E402
import concourse.bass as bass  # noqa: E402
import concourse.mybir as mybir  # noqa: E402
import concourse.tile as tile  # noqa: E402
from concourse.bass_utils import run_bass_kernel_spmd  # noqa: E402

E = 8
KTOP = 2
D = 768
I = 3072
B, S = 2, 2048
T = B * S
C = 1088          # per-expert token capacity (padded)
NCHUNK = 272      # moving free dim per matmul (<= 512 fp32 psum bank)
NCH = C // NCHUNK
DK = D // 128     # 6 contraction tiles for matmul 1
IK = I // 128     # 24 contraction tiles for matmul 2
N_CORES = 8
N_WARMUP_MM = 6   # dummy matmuls to ramp the PE HAM clock during DMA wait
W1BLK = 512       # W1 column-block tile width (streams in consumption order)
W1NB = I // W1BLK

MM_DT = mybir.dt.bfloat16
MM_NP = ml_dtypes.bfloat16

# Module-level knobs for test harness introspection.
TRACE = False
LAST_RESULT = None


def build_nc(act_func=None):
    """Build + compile the per-core Bass program (same program on all 8
    cores; per-core data differs)."""
    if act_func is None:
        act_func = mybir.ActivationFunctionType.Gelu

    nc = bacc.Bacc(
        "TRN2",
        target_bir_lowering=False,
        debug=False,
        enable_asserts=True,
        num_devices=N_CORES,
    )

    xT = nc.dram_tensor("xT", [D, C], MM_DT, kind="ExternalInput").ap()
    W1 = nc.dram_tensor("W1", [D, I], MM_DT, kind="ExternalInput").ap()
    W2 = nc.dram_tensor("W2", [I, D], MM_DT, kind="ExternalInput").ap()
    b1t = nc.dram_tensor("b1t", [128, IK], mybir.dt.float32, kind="ExternalInput").ap()
    b2t = nc.dram_tensor("b2t", [128, DK], mybir.dt.float32, kind="ExternalInput").ap()
    gb = nc.dram_tensor("gb", [128, C], mybir.dt.float32, kind="ExternalInput").ap()
    yT = nc.dram_tensor("yT", [D, C], mybir.dt.float32, kind="ExternalOutput").ap()

    with tile.TileContext(nc) as tc:
        with (
            tc.tile_pool(name="wpool", bufs=1) as wpool,
            tc.tile_pool(name="xpool", bufs=1) as xpool,
            tc.tile_pool(name="hpool", bufs=1) as hpool,
            tc.tile_pool(name="ypool", bufs=4) as ypool,
            tc.tile_pool(name="psum", bufs=8, space="PSUM") as psum_pool,
        ):
            # ---- PE warmup: ramp the HAM clock gate while DMAs land --------
            wu_w = wpool.tile([128, 128], MM_DT, name="wu_w", tag="wu_w")
            wu_r = wpool.tile([128, 512], MM_DT, name="wu_r", tag="wu_r")
            nc.gpsimd.memset(wu_w[:], 0.0)
            nc.gpsimd.memset(wu_r[:], 0.0)
            wu_ps = psum_pool.tile([128, 512], mybir.dt.float32,
                                   name="wu_ps", tag="ps")
            for _ in range(N_WARMUP_MM):
                nc.tensor.matmul(wu_ps[:], wu_w[:], wu_r[:], start=True, stop=True)

            # ---- resident loads --------------------------------------------
            # Issue in consumption order. Nearly everything goes on the Sync
            # HWDGE ring: the Scalar ring shares the Scalar engine with the
            # stage-1 gelus that recycle PSUM banks, so queueing bulk DMA
            # issues there stalls the whole pipeline. Scalar only carries the
            # six critical first W1 blocks (so they don't queue behind x).
            # W2/g load last and hide behind stage-1 compute.
            xsb, w1sb = [], [[None] * W1NB for _ in range(DK)]
            for kd in range(DK):
                tx = xpool.tile([128, C], MM_DT, name=f"x_{kd}", tag=f"x_{kd}")
                nc.sync.dma_start(tx[:], xT[kd * 128:(kd + 1) * 128, :])
                xsb.append(tx)
                tw = wpool.tile([128, W1BLK], MM_DT,
                                name=f"w1_{kd}_0", tag=f"w1_{kd}_0")
                nc.scalar.dma_start(tw[:], W1[kd * 128:(kd + 1) * 128, 0:W1BLK])
                w1sb[kd][0] = tw
            b1sb = wpool.tile([128, IK], mybir.dt.float32, name="b1sb", tag="b1sb")
            nc.sync.dma_start(b1sb[:], b1t[:])
            for j in range(1, W1NB):
                for kd in range(DK):
                    tw = wpool.tile([128, W1BLK], MM_DT,
                                    name=f"w1_{kd}_{j}", tag=f"w1_{kd}_{j}")
                    nc.sync.dma_start(tw[:], W1[kd * 128:(kd + 1) * 128,
                                                j * W1BLK:(j + 1) * W1BLK])
                    w1sb[kd][j] = tw
            w2sb = []
            for ki in range(IK):
                t = wpool.tile([128, D], MM_DT, name=f"w2_{ki}", tag=f"w2_{ki}")
                nc.sync.dma_start(t[:], W2[ki * 128:(ki + 1) * 128, :])
                w2sb.append(t)
            b2sb = wpool.tile([128, DK], mybir.dt.float32, name="b2sb", tag="b2sb")
            nc.sync.dma_start(b2sb[:], b2t[:])
            gsb = xpool.tile([128, C], mybir.dt.float32, name="gsb", tag="gsb")
            nc.sync.dma_start(gsb[:], gb[:])

            IPB = W1BLK // 128  # i-tiles per W1 block

            def w1_block(kd, i):
                return w1sb[kd][i // IPB][:, (i % IPB) * 128:(i % IPB + 1) * 128]

            # ---- stage 1: hT[i] = gelu(sum_d W1[d,i].T @ xT[d] + b1[i]) ----
            hsb = [
                hpool.tile([128, C], MM_DT, name=f"h_{i}", tag=f"h_{i}")
                for i in range(IK)
            ]
            for i in range(IK):
                for c in range(NCH):
                    ps = psum_pool.tile(
                        [128, NCHUNK], mybir.dt.float32,
                        name=f"ps1_{i}_{c}", tag="ps",
                    )
                    for d in range(DK):
                        nc.tensor.matmul(
                            ps[:],
                            w1_block(d, i),
                            xsb[d][:, c * NCHUNK:(c + 1) * NCHUNK],
                            start=(d == 0),
                            stop=(d == DK - 1),
                        )
                    nc.scalar.activation(
                        hsb[i][:, c * NCHUNK:(c + 1) * NCHUNK],
                        ps[:],
                        act_func,
                        bias=b1sb[:, i:i + 1],
                    )

            # ---- stage 2: yT[d] = (sum_ki W2[ki,d].T @ hT[ki] + b2[d]) * g --
            for dd in range(DK):
                for c in range(NCH):
                    ps = psum_pool.tile(
                        [128, NCHUNK], mybir.dt.float32,
                        name=f"ps2_{dd}_{c}", tag="ps",
                    )
                    for ki in range(IK):
                        nc.tensor.matmul(
                            ps[:],
                            w2sb[ki][:, dd * 128:(dd + 1) * 128],
                            hsb[ki][:, c * NCHUNK:(c + 1) * NCHUNK],
                            start=(ki == 0),
                            stop=(ki == IK - 1),
                        )
                    yt = ypool.tile(
                        [128, NCHUNK], mybir.dt.float32, name=f"y_{dd}_{c}", tag="y"
                    )
                    nc.vector.scalar_tensor_tensor(
                        yt[:],
                        ps[:],
                        b2sb[:, dd:dd + 1],
                        gsb[:, c * NCHUNK:(c + 1) * NCHUNK],
                        mybir.AluOpType.add,
                        mybir.AluOpType.mult,
                    )
                    nc.sync.dma_start(
                        yT[dd * 128:(dd + 1) * 128, c * NCHUNK:(c + 1) * NCHUNK],
                        yt[:],
                    )

    nc.compile()
    return nc


_COMPILED_NC = None


def _get_nc():
    global _COMPILED_NC
    if _COMPILED_NC is None:
        _COMPILED_NC = build_nc()
    return _COMPILED_NC


def _route(xf, Wr, br):
    """Router: logits -> softmax -> top-2. Uses jax on CPU so it is
    bit-identical to the reference; numpy fallback otherwise."""
    try:
        import jax
        import jax.numpy as jnp

        cpu = jax.devices("cpu")[0]
        with jax.default_device(cpu):
            logits = jnp.asarray(xf) @ jnp.asarray(Wr) + jnp.asarray(br)
            gates = jax.nn.softmax(logits, axis=-1)
            top_g, top_i = jax.lax.top_k(gates, KTOP)
        return np.asarray(top_g), np.asarray(top_i)
    except Exception:
        logits = xf @ np.asarray(Wr, np.float32) + np.asarray(br, np.float32)
        m = logits.max(axis=-1, keepdims=True)
        eg = np.exp(logits - m)
        gates = eg / eg.sum(axis=-1, keepdims=True)
        top_i = np.argsort(-gates, axis=-1, kind="stable")[:, :KTOP]
        top_g = np.take_along_axis(gates, top_i, axis=-1)
        return top_g.astype(np.float32), top_i.astype(np.int32)


def _host_expert(xg, W1e, b1e, W2e, b2e):
    """Exact fp32 expert FFN on host (overflow fallback only)."""
    h = xg @ W1e + b1e
    try:
        import jax

        h = np.asarray(jax.nn.gelu(h, approximate=False))
    except Exception:
        import math

        erf = np.vectorize(math.erf)
        h = 0.5 * h * (1.0 + erf(h / np.sqrt(2.0)))
    return h @ W2e + b2e


def kernel(x, W1, b1, W2, b2, Wr, br):
    global LAST_RESULT

    x = np.asarray(x, np.float32)
    W1 = np.asarray(W1, np.float32)
    b1 = np.asarray(b1, np.float32)
    W2 = np.asarray(W2, np.float32)
    b2 = np.asarray(b2, np.float32)
    Wr = np.asarray(Wr, np.float32)
    br = np.asarray(br, np.float32)

    xf = x.reshape(T, D)
    top_g, top_i = _route(xf, Wr, br)

    idxs, overflow = [], []
    in_maps = []
    for e in range(E):
        tok, kk = np.where(top_i == e)
        g = top_g[tok, kk].astype(np.float32)
        if len(tok) > C:
            overflow.append((e, tok[C:], g[C:]))
            tok, g = tok[:C], g[:C]
        idxs.append(tok)
        n = len(tok)

        xTg = np.zeros((D, C), MM_NP)
        xTg[:, :n] = xf[tok].T.astype(MM_NP)
        gbc = np.zeros((128, C), np.float32)
        gbc[:, :n] = g[None, :]
        in_maps.append({
            "xT": xTg,
            "W1": W1[e].astype(MM_NP),
            "W2": W2[e].astype(MM_NP),
            "b1t": np.ascontiguousarray(b1[e].reshape(IK, 128).T.astype(np.float32)),
            "b2t": np.ascontiguousarray(b2[e].reshape(DK, 128).T.astype(np.float32)),
            "gb": gbc,
        })

    nc = _get_nc()
    res = run_bass_kernel_spmd(nc, in_maps, list(range(N_CORES)), trace=TRACE)
    LAST_RESULT = res

    out = np.zeros((T, D), np.float32)
    for e in range(E):
        yTe = np.asarray(res.results[e]["yT"])  # [D, C] fp32
        n = len(idxs[e])
        if n:
            out[idxs[e]] += yTe[:, :n].T
    for e, tok, g in overflow:
        y = _host_expert(xf[tok], W1[e], b1[e], W2[e], b2[e])
        out[tok] += g[:, None] * y

    return out.reshape(B, S, D)
